# revision 32
# baseline (speedup 1.0000x reference)
"""Multi-head causal attention (B=2, S=2048, D=1024, H=16, HD=64) on 8 TRN2 cores.

Sharding: core c handles batch b = c//4 and heads 4*(c%4)..4*(c%4)+3.
The reference reshapes [b,h,s,hd] -> [b,s,1024] WITHOUT head transpose-back,
so output rows [128h, 128h+128) of y[b] depend only on head h: each core
produces a disjoint [512, 1024] block of the output. No collectives.

v3 (bf16 + trim + skewed pipeline + phase interleave):
  - All weight/activation DRAM inputs in bf16 (host-converted).
  - Diagonal score tiles trimmed: scores matmul, exp, and PV only cover
    q-cols >= 128r of the 512-q block; the partial 128x128 triangle block is
    masked post-exp by one strided bf16 DVE multiply (both heads at once).
  - Attention inner loop is software-pipelined: scores(t+1) is emitted
    before PV(t) so the in-order PE queue never serializes behind exp(t).
  - Pair-1 Q/K projections and pair-0 normalize/projection are emitted as
    fillers inside the opposite pair's attention j-loop, placed right after
    the second scores emission (where PE would otherwise stall on exp).
  - Normalization is per-j: denominators ride the PV matmul as a 65th V
    column, are DMA-gathered into dall rows (2j+q), reciprocal'd per j-block,
    broadcast via one-hot matmul, applied by DVE/Pool multiplies.
  - Output projection at K=128: attnT2b[h] is [128, 2048] with partitions
    64:128 holding a 1-col-left-shifted copy of rows 0:64 (SBUF->SBUF DMA),
    so lhsT [128,128] packs head-chunk pairs (m, m+1) and Wo contracts in 8
    chunks of 128 instead of 16 of 64 (halves proj PE rows).
"""

import sys

if "/opt/trn_rl_repo" not in sys.path:
    sys.path.insert(0, "/opt/trn_rl_repo")

from contextlib import ExitStack

import numpy as np
import ml_dtypes

import concourse.bass as bass
import concourse.tile as tile
from concourse import bacc, mybir
from concourse.masks import make_identity

F32 = mybir.dt.float32
F32R = mybir.dt.float32r
BF16 = mybir.dt.bfloat16
FP8 = mybir.dt.float8e4
EXP = mybir.ActivationFunctionType.Exp
DR = mybir.MatmulPerfMode.DoubleRow

B, S, D, H, HD = 2, 2048, 1024, 16, 64
NC = 8
HPC = 4  # heads per core
CT = D // 128  # 8 contraction tiles
QB = 4  # q-blocks of 512
KT = S // 128  # 16 k-tiles
WS = 64.0  # fp8 weight pre-scale (host); Q,K,V carry x64 out of projections
SCALE = 1.0 / (8.0 * WS * WS)  # exp scale absorbs Q*K fp8 scaling
NPBF16 = ml_dtypes.bfloat16
NPFP8 = ml_dtypes.float8_e4m3
DEBUG_DUMP = False


def build_nc():
    nc = bacc.Bacc("TRN2", target_bir_lowering=False, debug=False)

    xt = nc.dram_tensor("xt", [128, CT, S], FP8, kind="ExternalInput").ap()
    rxt = nc.dram_tensor("rxt", [128, CT, S], FP8, kind="ExternalInput").ap()
    wq = nc.dram_tensor("wq", [128, 2, CT, 128], FP8, kind="ExternalInput").ap()
    rwq = nc.dram_tensor("rwq", [128, 2, CT, 128], FP8, kind="ExternalInput").ap()
    wk = nc.dram_tensor("wk", [128, 2, CT, 128], FP8, kind="ExternalInput").ap()
    rwk = nc.dram_tensor("rwk", [128, 2, CT, 128], FP8, kind="ExternalInput").ap()
    wv = nc.dram_tensor("wv", [128, CT, 256], FP8, kind="ExternalInput").ap()
    rwv = nc.dram_tensor("rwv", [128, CT, 256], FP8, kind="ExternalInput").ap()
    wo2 = nc.dram_tensor("wo2", [128, 8, 4, 256], BF16, kind="ExternalInput").ap()
    bo = nc.dram_tensor("bo", [D], BF16, kind="ExternalInput").ap()
    masks = nc.dram_tensor("masks", [128, 2, 128], BF16, kind="ExternalInput").ap()
    oneh = nc.dram_tensor("oneh", [2, 128], BF16, kind="ExternalInput").ap()
    y = nc.dram_tensor("y", [HPC * 128, D], F32, kind="ExternalOutput").ap()
    dbg = (
        nc.dram_tensor("dbg", [128, S], BF16, kind="ExternalOutput").ap()
        if DEBUG_DUMP
        else None
    )
    dbg2 = (
        nc.dram_tensor("dbg2", [128, 260], F32, kind="ExternalOutput").ap()
        if DEBUG_DUMP
        else None
    )

    with tile.TileContext(nc) as tc, ExitStack() as ctx:
        a_pool = ctx.enter_context(tc.tile_pool(name="a", bufs=1))

        # ---- resident SBUF tensors
        xt_sb = a_pool.tile([128, CT, S], FP8, tag="xt")
        rxt_sb = a_pool.tile([128, CT, S], FP8, tag="rxt")
        wq_sb = a_pool.tile([128, 2, CT, 128], FP8, tag="wq")
        rwq_sb = a_pool.tile([128, 2, CT, 128], FP8, tag="rwq")
        wk_sb = a_pool.tile([128, 2, CT, 128], FP8, tag="wk")
        rwk_sb = a_pool.tile([128, 2, CT, 128], FP8, tag="rwk")
        wv_sb = a_pool.tile([128, CT, 256], FP8, tag="wv")
        rwv_sb = a_pool.tile([128, CT, 256], FP8, tag="rwv")
        wo2_sb = a_pool.tile([128, 8, 4, 256], BF16, tag="wo2")
        masks_sb = a_pool.tile([128, 2, 128], BF16, tag="masks")
        oneh_sb = a_pool.tile([2, 128], BF16, tag="oneh")
        bo_sb = a_pool.tile([128, D], BF16, tag="bo")
        # V packed [128(s_local), 16 s-tiles, 4*(64+ones col)] bf16
        v4 = a_pool.tile([128, KT, 260], BF16, tag="v4")
        qst = [a_pool.tile([128, S], BF16, tag=f"qst{p}", name=f"qst{p}") for p in range(2)]
        kst = [a_pool.tile([128, S], BF16, tag=f"kst{p}", name=f"kst{p}") for p in range(2)]
        qtb = [a_pool.tile([64, S], BF16, tag=f"qtb{p}", name=f"qtb{p}") for p in range(2)]
        ktb = [a_pool.tile([64, S], BF16, tag=f"ktb{p}", name=f"ktb{p}") for p in range(2)]
        # attnT2b[h]: rows 0:64 = attn^T (hd x q), rows 64:128 = 1-col-left-
        # shifted copy (for K=128 proj lhsT)
        attnT2b = [
            a_pool.tile([128, S], BF16, tag=f"at{h}", name=f"at{h}") for h in range(HPC)
        ]
        # identity for PE transposes (attn [q, hd] -> attn^T [hd, q]); the
        # shifted identity (ident_s[q, m] = 1 iff q == m+1) produces the
        # one-col-left-shifted duplicate rows directly from a transpose
        ident = a_pool.tile([128, 128], BF16, tag="ident")
        make_identity(nc, ident[:])
        ident_s = a_pool.tile([128, 128], BF16, tag="ident_s")
        nc.gpsimd.memset(ident_s[:], 0.0)
        nc.gpsimd.affine_select(
            out=ident_s[:],
            in_=ident_s[:],
            compare_op=mybir.AluOpType.not_equal,
            fill=1.0,
            base=-1,
            # fill where (x - y - 1) == 0, i.e. partition q = col m + 1
            pattern=[[-1, 128]],
            channel_multiplier=1,
        )

        # warm up the Act engine's Exp table at t~0 (it otherwise lazy-loads
        # 1.3us right at the first attention exp)
        ones_col = a_pool.tile([1, 128], BF16, tag="ones_col")
        nc.vector.memset(ones_col[:], 1.0)
        warm = a_pool.tile([1, 8], F32, tag="warm")
        warm2 = a_pool.tile([1, 8], F32, tag="warm2")
        nc.vector.memset(warm[:], 0.0)
        nc.scalar.activation(warm2[:], warm[:], EXP, scale=SCALE)

        # ---- input DMAs (SP queue; order = need order; wo2 issued after P1)
        nc.sync.dma_start(out=wq_sb[:, 0, 0:2], in_=wq[:, 0, 0:2])
        nc.sync.dma_start(out=wk_sb[:, 0, 0:2], in_=wk[:, 0, 0:2])
        nc.sync.dma_start(out=xt_sb[:, 0:2, :], in_=xt[:, 0:2, :])
        nc.sync.dma_start(out=wq_sb[:, 0, 2:8], in_=wq[:, 0, 2:8])
        nc.sync.dma_start(out=wk_sb[:, 0, 2:8], in_=wk[:, 0, 2:8])
        for quad in range(1, 4):
            nc.sync.dma_start(
                out=xt_sb[:, 2 * quad : 2 * quad + 2, :],
                in_=xt[:, 2 * quad : 2 * quad + 2, :],
            )
        nc.sync.dma_start(out=rxt_sb[:], in_=rxt)
        nc.sync.dma_start(out=rwq_sb[:, 0], in_=rwq[:, 0])
        nc.sync.dma_start(out=rwk_sb[:, 0], in_=rwk[:, 0])
        nc.sync.dma_start(out=wv_sb[:], in_=wv)
        nc.sync.dma_start(out=rwv_sb[:], in_=rwv)
        nc.sync.dma_start(out=wq_sb[:, 1], in_=wq[:, 1])
        nc.sync.dma_start(out=wk_sb[:, 1], in_=wk[:, 1])
        nc.sync.dma_start(out=rwq_sb[:, 1], in_=rwq[:, 1])
        nc.sync.dma_start(out=rwk_sb[:, 1], in_=rwk[:, 1])
        nc.sync.dma_start(out=masks_sb[:], in_=masks)
        nc.sync.dma_start(out=oneh_sb[:], in_=oneh)
        bo_b = bass.AP(tensor=bo.tensor, offset=bo.offset, ap=[[0, 128], [1, D]])
        nc.sync.dma_start(out=bo_sb[:], in_=bo_b)
        # ones column of v4 via memset (strided view)
        nc.gpsimd.memset(
            v4[:].rearrange("p t (h c) -> p t h c", c=65)[:, :, :, 64:65], 1.0
        )

        y_pool = ctx.enter_context(tc.tile_pool(name="y", bufs=6))
        rc_pool = ctx.enter_context(tc.tile_pool(name="rc", bufs=4))
        an_pool = ctx.enter_context(tc.tile_pool(name="an", bufs=4))
        pt_pool = ctx.enter_context(tc.tile_pool(name="pt", bufs=3))

        # ---- P1 pair 0: Q/K fp8 DoubleRow 3-term (x8.W8 + rx8.W8 + x8.rW8)
        # with 8 live psum accumulators, then V (same 3-term scheme)
        qk_terms = (
            (xt_sb, wq_sb, wk_sb),
            (rxt_sb, wq_sb, wk_sb),
            (xt_sb, rwq_sb, rwk_sb),
        )
        with ExitStack() as scope1:
            ps1 = scope1.enter_context(tc.tile_pool(name="ps1", bufs=2, space="PSUM"))
            psqk = [
                ps1.tile([128, 512], F32, tag=f"qk{i}", name=f"qk{i}", bufs=1)
                for i in range(8)
            ]
            for term, (xsrc, wq_t, wk_t) in enumerate(qk_terms):
                for cp in range(4):
                    for i, w_sb in ((0, wq_t), (4, wk_t)):
                        for nb in range(QB):
                            nc.tensor.matmul(
                                psqk[i + nb][:],
                                w_sb[:, 0, 2 * cp : 2 * cp + 2, :],
                                xsrc[:, 2 * cp : 2 * cp + 2, bass.ts(nb, 512)],
                                start=(term == 0 and cp == 0),
                                stop=(term == 2 and cp == 3),
                                perf_mode=DR,
                            )
            for i, dst in ((0, qst[0]), (4, kst[0])):
                for nb in range(QB):
                    nc.vector.tensor_copy(dst[:, bass.ts(nb, 512)], psqk[i + nb][:])
            nc.sync.dma_start(out=qtb[0][:], in_=qst[0][64:128, :])
            nc.sync.dma_start(out=ktb[0][:], in_=kst[0][64:128, :])
            nc.sync.dma_start(out=wo2_sb[:], in_=wo2)
            # V for all 4 heads (st-outer, 3-term fp8 ct accumulation)
            for st in range(KT):
                ps = ps1.tile([128, 256], F32, tag=f"qk{st % 8}", name="psv", bufs=1)
                for term, (xsrc, _, _) in enumerate(qk_terms):
                    wv_t = rwv_sb if term == 2 else wv_sb
                    for cp in range(4):
                        nc.tensor.matmul(
                            ps[:],
                            xsrc[:, 2 * cp : 2 * cp + 2, bass.ts(st, 128)],
                            wv_t[:, 2 * cp : 2 * cp + 2, :],
                            start=(term == 0 and cp == 0),
                            stop=(term == 2 and cp == 3),
                            perf_mode=DR,
                        )
                nc.scalar.activation(
                    v4[:, st, :].rearrange("p (h c) -> p h c", c=65)[:, :, 0:64],
                    ps[:].rearrange("p (h c) -> p h c", c=64),
                    mybir.ActivationFunctionType.Copy,
                    scale=1.0 / WS,
                )

        # ---- P2: attention, software-pipelined, with interleaved fillers
        ps2 = ctx.enter_context(tc.tile_pool(name="ps2", bufs=2, space="PSUM"))

        def attn_scores(p, j, t, qv):
            """scores^T both heads -> exp -> mask (diagonal). Returns pt2."""
            r = t - 4 * j
            c0 = 128 * r if r > 0 else 0
            pss = ps2.tile([128, 1024], F32, tag="pss", name="pss", bufs=2)
            for q in range(2):
                qt, kt = qv[q]
                nc.tensor.matmul(
                    pss[:, 512 * q + c0 : 512 * (q + 1)],
                    kt[:, bass.ts(t, 128)],
                    qt[:, 512 * j + c0 : 512 * (j + 1)],
                    start=True,
                    stop=True,
                )
            pt2 = pt_pool.tile([128, 1024], BF16, tag="pt2", name="pt2")
            if r < 0:
                nc.scalar.activation(pt2[:], pss[:], EXP, scale=SCALE)
            else:
                pv = pss[:].rearrange("p (h c) -> p h c", c=512)[:, :, c0:512]
                ov = pt2[:].rearrange("p (h c) -> p h c", c=512)[:, :, c0:512]
                nc.scalar.activation(ov, pv, EXP, scale=SCALE)
                mv = pt2[:].rearrange("p (h c) -> p h c", c=512)[:, :, c0 : c0 + 128]
                nc.vector.tensor_mul(mv, mv, masks_sb[:])
            return pt2

        def attn_pv(p, j, t, pt2, psa):
            """PV in [q, hd] orientation: lhsT = P^T tile (stationary 128 q),
            rhs = V (+ones col) moving 65. psa[q] = [128 q, 4 qt x 65]."""
            r = t - 4 * j
            for q in range(2):
                h = 2 * p + q
                vsl = v4[:, t, bass.ds(65 * h, 65)]
                for qt in range(max(r, 0), 4):
                    # start only on the bank's very first matmul: start=True
                    # clears the has_written bits of the WHOLE 2KB zero
                    # region, so a second start would wipe sibling qt
                    # regions' accumulate state
                    nc.tensor.matmul(
                        psa[q][:, 65 * qt : 65 * qt + 65],
                        pt2[:, 512 * q + 128 * qt : 512 * q + 128 * qt + 128],
                        vsl,
                        start=(t == 0 and qt == 0),
                        stop=(t == 4 * j + qt),
                    )

        def norm_transpose(p, j, psa, shift_by_copy=False, keepers=False):
            if DEBUG_DUMP and p == 0 and j == 0:
                dbg2_sb = a_pool.tile([128, 260], F32, tag="dbg2sb")
                nc.vector.tensor_copy(dbg2_sb[:], psa[0][:])
                nc.sync.dma_start(out=dbg2, in_=dbg2_sb[:])
            """Reciprocal of the softmax denominators (col 64 of each 65-block
            of psa), per-partition-scalar normalize psum->sbuf bf16, PE
            transpose back to [hd, q], land both halves in attnT2b.
            shift_by_copy: emit the odd-m shifted duplicate as a second
            psum->sbuf copy instead of a SBUF DMA (used in the tail where the
            DMA's ~1.6us latency would sit on the critical path)."""
            rcps, ans, psts = [], [], []
            if keepers:
                # immediate ramp-keepers cover the recip latency window
                dmy = ps2.tile([128, 512], F32, tag="pst0", name="dmy", bufs=1)
                for _ in range(4):
                    nc.tensor.matmul(
                        dmy[0:4, 0:128], oneh_sb[:, 0:4], oneh_sb[:],
                        start=True, stop=True,
                    )
            for q in range(2):
                rcp = rc_pool.tile([128, 4], F32, tag="rc", name="rc")
                dn = psa[q][:].rearrange("p (qt c) -> p qt c", c=65)[:, :, 64:65]
                with nc.allow_low_precision(reason="softmax denom reciprocal"):
                    nc.vector.reciprocal(rcp[:], dn)
                rcps.append(rcp)
            for q in range(2):
                an = an_pool.tile([128, 256], BF16, tag="an", name="an")
                for qt in range(4):
                    nc.vector.tensor_scalar_mul(
                        an[:, 64 * qt : 64 * qt + 64],
                        psa[q][:, 65 * qt : 65 * qt + 64],
                        rcps[q][:, qt : qt + 1],
                    )
                ans.append(an)
                if keepers and q == 0:
                    # gated keepers: ready right as the q0 muls complete,
                    # keeping PE warm until the transposes are unblocked
                    for _ in range(6):
                        nc.tensor.matmul(
                            dmy[0:4, 0:128], an[:, 0:4], ident[:],
                            start=True, stop=True,
                        )
            for q in range(2):
                rows = 128 if shift_by_copy else 64
                pst = ps2.tile([rows, 512], BF16, tag=f"pst{q}", name=f"pst{q}", bufs=1)
                for qt in range(4):
                    nc.tensor.transpose(
                        pst[0:64, 128 * qt : 128 * qt + 128],
                        ans[q][:, 64 * qt : 64 * qt + 64],
                        ident[:],
                    )
                    if shift_by_copy:
                        # shifted duplicate straight from a second transpose
                        # (within-tile shift; odd cols 127/255/... land as 0,
                        # never read by the proj's even-m access pattern)
                        nc.tensor.transpose(
                            pst[64:128, 128 * qt : 128 * qt + 128],
                            ans[q][:, 64 * qt : 64 * qt + 64],
                            ident_s[:],
                        )
                psts.append(pst)
            for q in range(2):
                h = 2 * p + q
                if shift_by_copy:
                    nc.vector.tensor_copy(
                        attnT2b[h][:, bass.ts(j, 512)], psts[q][:]
                    )
                else:
                    nc.vector.tensor_copy(
                        attnT2b[h][0:64, bass.ts(j, 512)], psts[q][:]
                    )
                    nc.sync.dma_start(
                        out=attnT2b[h][64:128, 512 * j : 512 * j + 511],
                        in_=attnT2b[h][0:64, 512 * j + 1 : 512 * (j + 1)],
                    )

        def attn_j(p, j, qv, slot0=(), slot1=(), slot2=(), pre=None):
            """Skewed pipeline: S0 S1 [pre] [slot0] P0 S2 [slot1] P1 S3 ... PT.
            Each slot must stay under ~2us of PE work (the Act runway from
            the two queued exps) or Act starves behind the in-order PE queue.
            Returns a closure finishing this j (normalize+transpose), which
            the caller passes as `pre` to the NEXT attn_j so the next block's
            first scores are not queued behind it."""
            T = 4 * j + 4
            psa = [
                ps2.tile([128, 260], F32, tag=f"psa{q}", name=f"psa{q}", bufs=1)
                for q in range(2)
            ]
            pts = [attn_scores(p, j, 0, qv)]
            if T > 1:
                pts.append(attn_scores(p, j, 1, qv))
            if pre is not None:
                pre()
            for f in slot0:
                f()
            for t in range(T):
                if t + 2 < T:
                    pts.append(attn_scores(p, j, t + 2, qv))
                if t == 1:
                    for f in slot1:
                        f()
                if t == 5:
                    for f in slot2:
                        f()
                attn_pv(p, j, t, pts[t], psa)

            def finish():
                norm_transpose(p, j, psa)

            return finish, psa

        def qk1_half(half, nb):
            """Pair-1 Q (half=0) or K (half=1) fp8 3-term projection for one
            512-col s-block (rides pss tag; ~1.3us of PE)."""
            w_sb, rw_sb, dst, dstb = (
                (wq_sb, rwq_sb, qst[1], qtb[1]),
                (wk_sb, rwk_sb, kst[1], ktb[1]),
            )[half]
            pssqk = ps2.tile([128, 1024], F32, tag="pss", name="pssqk", bufs=2)
            for term, (xsrc, wt) in enumerate(
                ((xt_sb, w_sb), (rxt_sb, w_sb), (xt_sb, rw_sb))
            ):
                for cp in range(4):
                    nc.tensor.matmul(
                        pssqk[:, 0:512],
                        wt[:, 1, 2 * cp : 2 * cp + 2, :],
                        xsrc[:, 2 * cp : 2 * cp + 2, bass.ts(nb, 512)],
                        start=(term == 0 and cp == 0),
                        stop=(term == 2 and cp == 3),
                        perf_mode=DR,
                    )
            nc.vector.tensor_copy(dst[:, bass.ts(nb, 512)], pssqk[:, 0:512])
            nc.sync.dma_start(
                out=dstb[:, bass.ts(nb, 512)], in_=dst[64:128, bass.ts(nb, 512)]
            )

        def proj_qq(h, qq0, tail=False):
            """Project head h for quarter pair (qq0, qq0+1); one psum tile
            holds both quarters so the pss-tag rotation stalls half as often.
            In the tail, ys adds and y DMA issues are spread across engines
            so the final store ladder does not serialize on one queue."""
            a2 = attnT2b[h][:].rearrange("p (r s) -> p s r", s=16)
            psy = ps2.tile([128, 1024], F32, tag="pss", name="psy", bufs=2)
            for k in range(2):
                qq = qq0 + k
                if tail and k == 1:
                    # pre-add bo into psum so the psum->sbuf move can be an
                    # Act-engine Copy (Act is idle in the tail; DVE is not)
                    nc.tensor.matmul(
                        psy[:, bass.ts(k, 256)],
                        ones_col[:],
                        bo_sb[0:1, bass.ts(qq, 256)],
                        start=True,
                        stop=False,
                    )
                for mp in range(8):
                    nc.tensor.matmul(
                        psy[:, bass.ts(k, 256)],
                        a2[:, 2 * mp, :],
                        wo2_sb[:, mp, qq, :],
                        start=(mp == 0) and not (tail and k == 1),
                        stop=(mp == 7),
                    )
            for k in range(2):
                qq = qq0 + k
                ys = y_pool.tile([128, 256], F32, tag="ys", name="ys")
                if tail and k == 1:
                    nc.scalar.activation(
                        ys[:], psy[:, bass.ts(k, 256)],
                        mybir.ActivationFunctionType.Copy,
                    )
                else:
                    nc.vector.tensor_add(
                        ys[:], psy[:, bass.ts(k, 256)], bo_sb[:, bass.ts(qq, 256)]
                    )
                nc.sync.dma_start(out=y[bass.ts(h, 128), bass.ts(qq, 256)], in_=ys[:])

        qv0 = [(qst[0][0:64, :], kst[0][0:64, :]), (qtb[0][:], ktb[0][:])]
        qv1 = [(qst[1][0:64, :], kst[1][0:64, :]), (qtb[1][:], ktb[1][:])]

        def Q1(nb):
            return lambda: qk1_half(0, nb)

        def K1(nb):
            return lambda: qk1_half(1, nb)

        def P(h, qq):
            return lambda: proj_qq(h, qq)

        # qk1 half-chunks (~1.3us each) are deadline-scheduled: attn1-j needs
        # qst[1]/kst[1] block nb=j only. Pair-0 projections fill pair-1's
        # attention (attnT2b[0..1] complete after finish(0,3) = pre of (1,0)).
        # Each j's finish (normalize+transpose) is deferred into the next j,
        # emitted behind its first two scores so the Act feed isn't delayed.
        fin, _ = attn_j(0, 0, qv0, [Q1(0)])
        fin, _ = attn_j(0, 1, qv0, [K1(0)], pre=fin)
        fin, _ = attn_j(0, 2, qv0, [Q1(1)], [K1(1)], pre=fin)
        fin, _ = attn_j(0, 3, qv0, [Q1(2)], [K1(2)], pre=fin)

        fin, _ = attn_j(1, 0, qv1, [Q1(3)], [P(0, 0)], pre=fin)
        fin, _ = attn_j(1, 1, qv1, [K1(3)], [P(0, 2)], pre=fin)
        fin, _ = attn_j(1, 2, qv1, [P(1, 0)], [P(1, 2)], pre=fin)
        fin, psa13 = attn_j(1, 3, qv1, pre=fin)

        # tail: finish pair-1 last j inline (shifted dup via psum copies so no
        # DMA latency gates the projections), then heads 2,3 projections
        norm_transpose(1, 3, psa13, shift_by_copy=True, keepers=True)
        proj_qq(2, 0, tail=True)
        proj_qq(2, 2, tail=True)
        proj_qq(3, 0, tail=True)
        for qq in (2, 3):
            a2 = attnT2b[3][:].rearrange("p (r s) -> p s r", s=16)
            psy = ps2.tile([128, 1024], F32, tag="pss", name="psy1", bufs=2)
            for mp in range(8):
                nc.tensor.matmul(
                    psy[:, 0:256],
                    a2[:, 2 * mp, :],
                    wo2_sb[:, mp, qq, :],
                    start=(mp == 0),
                    stop=(mp == 7),
                )
            ys = y_pool.tile([128, 256], F32, tag="ys", name="ys")
            eng_d = (nc.scalar, nc.sync)[qq - 2]
            nc.vector.tensor_add(ys[:], psy[:, 0:256], bo_sb[:, bass.ts(qq, 256)])
            eng_d.dma_start(out=y[bass.ts(3, 128), bass.ts(qq, 256)], in_=ys[:])
        if DEBUG_DUMP:
            nc.sync.dma_start(out=dbg, in_=attnT2b[0][:])

    nc.compile()
    return nc


def make_masks():
    kl = np.arange(128)[:, None]
    cl = np.arange(128)[None, :]
    tri = (kl <= cl).astype(NPBF16)  # [128 k, 128 c]
    return np.ascontiguousarray(np.stack([tri, tri], 1))  # [128, 2, 128]


def fp8_split(a):
    """a (f32) -> (fp8(a), fp8(a - fp8(a))) both contiguous."""
    a8 = a.astype(NPFP8)
    r8 = (a - a8.astype(np.float32)).astype(NPFP8)
    return np.ascontiguousarray(a8), np.ascontiguousarray(r8)


def prep_core_inputs(c, x, Wq, Wk, Wv, Wo, bo):
    b, g = c // 4, c % 4
    heads = [4 * g + i for i in range(HPC)]
    xt, rxt = fp8_split(x[b].T.reshape(CT, 128, S).transpose(1, 0, 2))

    def pack_pair(W, p):
        h0, h1 = heads[2 * p], heads[2 * p + 1]
        cols = np.concatenate(
            [W[:, 64 * h0 : 64 * h0 + 64], W[:, 64 * h1 : 64 * h1 + 64]], 1
        )
        return cols.reshape(CT, 128, 128)

    wq, rwq = fp8_split(
        np.stack([pack_pair(Wq, p) for p in range(2)]).transpose(2, 0, 1, 3) * WS
    )  # [128, 2, CT, 128]
    wk, rwk = fp8_split(
        np.stack([pack_pair(Wk, p) for p in range(2)]).transpose(2, 0, 1, 3) * WS
    )
    wv, rwv = fp8_split(
        np.concatenate([Wv[:, 64 * h : 64 * h + 64] for h in heads], 1)
        .reshape(CT, 128, 256)
        .transpose(1, 0, 2)
        * WS
    )  # [128, CT, 256]
    # wo2[d, mp, qq, :] = Wo[128*mp + d, 256*qq : 256*(qq+1)]
    wo2 = np.ascontiguousarray(
        Wo.reshape(8, 128, 4, 256).transpose(1, 0, 2, 3)
    ).astype(NPBF16)  # [128, 8, 4, 256]
    oneh = np.kron(np.eye(2, dtype=np.float32), np.ones((1, 64), np.float32)).astype(
        NPBF16
    )  # [2, 128]
    return {
        "xt": xt,
        "rxt": rxt,
        "wq": wq,
        "rwq": rwq,
        "wk": wk,
        "rwk": rwk,
        "wv": wv,
        "rwv": rwv,
        "wo2": wo2,
        "bo": bo.astype(NPBF16),
        "masks": make_masks(),
        "oneh": oneh,
    }


_NC_CACHE = []


def kernel(x, Wq, Wk, Wv, Wo, bo):
    from concourse import bass_utils

    x, Wq, Wk, Wv, Wo, bo = (
        np.asarray(x, np.float32),
        np.asarray(Wq, np.float32),
        np.asarray(Wk, np.float32),
        np.asarray(Wv, np.float32),
        np.asarray(Wo, np.float32),
        np.asarray(bo, np.float32),
    )
    if not _NC_CACHE:
        _NC_CACHE.append(build_nc())
    nc = _NC_CACHE[0]
    in_maps = [prep_core_inputs(c, x, Wq, Wk, Wv, Wo, bo) for c in range(NC)]
    res = bass_utils.run_bass_kernel_spmd(nc, in_maps, core_ids=list(range(NC)))
    out = np.empty((B, S, D), np.float32)
    for c in range(NC):
        b, g = c // 4, c % 4
        out[b, 512 * g : 512 * (g + 1), :] = res.results[c]["y"]
    return out



# revision 35
# speedup vs baseline: 1.0920x; 1.0920x over previous
"""Multi-head causal attention (B=2, S=2048, D=1024, H=16, HD=64) on 8 TRN2 cores.

Sharding: core c handles batch b = c//4 and heads 4*(c%4)..4*(c%4)+3.
The reference reshapes [b,h,s,hd] -> [b,s,1024] WITHOUT head transpose-back,
so output rows [128h, 128h+128) of y[b] depend only on head h: each core
produces a disjoint [512, 1024] block of the output. No collectives.

v3 (bf16 + trim + skewed pipeline + phase interleave):
  - All weight/activation DRAM inputs in bf16 (host-converted).
  - Diagonal score tiles trimmed: scores matmul, exp, and PV only cover
    q-cols >= 128r of the 512-q block; the partial 128x128 triangle block is
    masked post-exp by one strided bf16 DVE multiply (both heads at once).
  - Attention inner loop is software-pipelined: scores(t+1) is emitted
    before PV(t) so the in-order PE queue never serializes behind exp(t).
  - Pair-1 Q/K projections and pair-0 normalize/projection are emitted as
    fillers inside the opposite pair's attention j-loop, placed right after
    the second scores emission (where PE would otherwise stall on exp).
  - Normalization is per-j: denominators ride the PV matmul as a 65th V
    column, are DMA-gathered into dall rows (2j+q), reciprocal'd per j-block,
    broadcast via one-hot matmul, applied by DVE/Pool multiplies.
  - Output projection at K=128: attnT2b[h] is [128, 2048] with partitions
    64:128 holding a 1-col-left-shifted copy of rows 0:64 (SBUF->SBUF DMA),
    so lhsT [128,128] packs head-chunk pairs (m, m+1) and Wo contracts in 8
    chunks of 128 instead of 16 of 64 (halves proj PE rows).
"""

import sys

if "/opt/trn_rl_repo" not in sys.path:
    sys.path.insert(0, "/opt/trn_rl_repo")

from contextlib import ExitStack

import numpy as np
import ml_dtypes

import concourse.bass as bass
import concourse.tile as tile
from concourse import bacc, mybir
from concourse.masks import make_identity

F32 = mybir.dt.float32
F32R = mybir.dt.float32r
BF16 = mybir.dt.bfloat16
FP8 = mybir.dt.float8e4
EXP = mybir.ActivationFunctionType.Exp
DR = mybir.MatmulPerfMode.DoubleRow

B, S, D, H, HD = 2, 2048, 1024, 16, 64
NC = 8
HPC = 4  # heads per core
CT = D // 128  # 8 contraction tiles
QB = 4  # q-blocks of 512
KT = S // 128  # 16 k-tiles
WS = 64.0  # fp8 weight pre-scale (host); Q,K,V carry x64 out of projections
SCALE = 1.0 / (8.0 * WS * WS)  # exp scale absorbs Q*K fp8 scaling
NPBF16 = ml_dtypes.bfloat16
NPFP8 = ml_dtypes.float8_e4m3
DEBUG_DUMP = False


def build_nc():
    nc = bacc.Bacc("TRN2", target_bir_lowering=False, debug=False)

    xt = nc.dram_tensor("xt", [128, CT, S], FP8, kind="ExternalInput").ap()
    rxt = nc.dram_tensor("rxt", [128, CT, S], FP8, kind="ExternalInput").ap()
    wq = nc.dram_tensor("wq", [128, 2, CT, 128], FP8, kind="ExternalInput").ap()
    rwq = nc.dram_tensor("rwq", [128, 2, CT, 128], FP8, kind="ExternalInput").ap()
    wk = nc.dram_tensor("wk", [128, 2, CT, 128], FP8, kind="ExternalInput").ap()
    rwk = nc.dram_tensor("rwk", [128, 2, CT, 128], FP8, kind="ExternalInput").ap()
    wv = nc.dram_tensor("wv", [128, CT, 256], FP8, kind="ExternalInput").ap()
    rwv = nc.dram_tensor("rwv", [128, CT, 256], FP8, kind="ExternalInput").ap()
    wo2 = nc.dram_tensor("wo2", [128, 8, 4, 256], BF16, kind="ExternalInput").ap()
    bo = nc.dram_tensor("bo", [D], BF16, kind="ExternalInput").ap()
    masks = nc.dram_tensor("masks", [128, 2, 128], BF16, kind="ExternalInput").ap()
    oneh = nc.dram_tensor("oneh", [2, 128], BF16, kind="ExternalInput").ap()
    y = nc.dram_tensor("y", [HPC * 128, D], F32, kind="ExternalOutput").ap()
    dbg = (
        nc.dram_tensor("dbg", [128, S], BF16, kind="ExternalOutput").ap()
        if DEBUG_DUMP
        else None
    )
    dbg2 = (
        nc.dram_tensor("dbg2", [128, 260], F32, kind="ExternalOutput").ap()
        if DEBUG_DUMP
        else None
    )

    with tile.TileContext(nc) as tc, ExitStack() as ctx:
        a_pool = ctx.enter_context(tc.tile_pool(name="a", bufs=1))

        # ---- resident SBUF tensors
        xt_sb = a_pool.tile([128, CT, S], FP8, tag="xt")
        rxt_sb = a_pool.tile([128, CT, S], FP8, tag="rxt")
        wq_sb = a_pool.tile([128, 2, CT, 128], FP8, tag="wq")
        rwq_sb = a_pool.tile([128, 2, CT, 128], FP8, tag="rwq")
        wk_sb = a_pool.tile([128, 2, CT, 128], FP8, tag="wk")
        rwk_sb = a_pool.tile([128, 2, CT, 128], FP8, tag="rwk")
        wv_sb = a_pool.tile([128, CT, 256], FP8, tag="wv")
        rwv_sb = a_pool.tile([128, CT, 256], FP8, tag="rwv")
        wo2_sb = a_pool.tile([128, 8, 4, 256], BF16, tag="wo2")
        masks_sb = a_pool.tile([128, 2, 128], BF16, tag="masks")
        oneh_sb = a_pool.tile([2, 128], BF16, tag="oneh")
        bo_sb = a_pool.tile([128, D], BF16, tag="bo")
        # V packed [128(s_local), 16 s-tiles, 4*(64+ones col)] bf16
        v4 = a_pool.tile([128, KT, 260], BF16, tag="v4")
        qst = [a_pool.tile([128, S], BF16, tag=f"qst{p}", name=f"qst{p}") for p in range(2)]
        kst = [a_pool.tile([128, S], BF16, tag=f"kst{p}", name=f"kst{p}") for p in range(2)]
        qtb = [a_pool.tile([64, S], BF16, tag=f"qtb{p}", name=f"qtb{p}") for p in range(2)]
        ktb = [a_pool.tile([64, S], BF16, tag=f"ktb{p}", name=f"ktb{p}") for p in range(2)]
        # attnT2b[h]: rows 0:64 = attn^T (hd x q), rows 64:128 = 1-col-left-
        # shifted copy (for K=128 proj lhsT)
        attnT2b = [
            a_pool.tile([128, S], BF16, tag=f"at{h}", name=f"at{h}") for h in range(HPC)
        ]
        # identity for PE transposes (attn [q, hd] -> attn^T [hd, q]); the
        # shifted identity (ident_s[q, m] = 1 iff q == m+1) produces the
        # one-col-left-shifted duplicate rows directly from a transpose
        ident = a_pool.tile([128, 128], BF16, tag="ident")
        make_identity(nc, ident[:])
        ident_s = a_pool.tile([128, 128], BF16, tag="ident_s")
        nc.gpsimd.memset(ident_s[:], 0.0)
        nc.gpsimd.affine_select(
            out=ident_s[:],
            in_=ident_s[:],
            compare_op=mybir.AluOpType.not_equal,
            fill=1.0,
            base=-1,
            # fill where (x - y - 1) == 0, i.e. partition q = col m + 1
            pattern=[[-1, 128]],
            channel_multiplier=1,
        )

        # warm up the Act engine's Exp table at t~0 (it otherwise lazy-loads
        # 1.3us right at the first attention exp)
        ones_col = a_pool.tile([1, 128], BF16, tag="ones_col")
        nc.vector.memset(ones_col[:], 1.0)
        warm = a_pool.tile([1, 8], F32, tag="warm")
        warm2 = a_pool.tile([1, 8], F32, tag="warm2")
        nc.vector.memset(warm[:], 0.0)
        nc.scalar.activation(warm2[:], warm[:], EXP, scale=SCALE)

        # ---- input DMAs (SP queue; order = need order; wo2 issued after P1)
        nc.sync.dma_start(out=wq_sb[:, 0, 0:2], in_=wq[:, 0, 0:2])
        nc.sync.dma_start(out=wk_sb[:, 0, 0:2], in_=wk[:, 0, 0:2])
        nc.sync.dma_start(out=xt_sb[:, 0:2, :], in_=xt[:, 0:2, :])
        nc.sync.dma_start(out=wq_sb[:, 0, 2:8], in_=wq[:, 0, 2:8])
        nc.sync.dma_start(out=wk_sb[:, 0, 2:8], in_=wk[:, 0, 2:8])
        # interleave xt/rxt chunks in QK-term need order (term a consumes xt
        # chunk-by-chunk; term b starts on rxt right after)
        nc.sync.dma_start(out=xt_sb[:, 2:4, :], in_=xt[:, 2:4, :])
        nc.sync.dma_start(out=xt_sb[:, 4:6, :], in_=xt[:, 4:6, :])
        nc.sync.dma_start(out=rxt_sb[:, 0:2, :], in_=rxt[:, 0:2, :])
        nc.sync.dma_start(out=xt_sb[:, 6:8, :], in_=xt[:, 6:8, :])
        nc.sync.dma_start(out=rxt_sb[:, 2:4, :], in_=rxt[:, 2:4, :])
        nc.sync.dma_start(out=rwq_sb[:, 0], in_=rwq[:, 0])
        nc.sync.dma_start(out=rwk_sb[:, 0], in_=rwk[:, 0])
        nc.sync.dma_start(out=rxt_sb[:, 4:6, :], in_=rxt[:, 4:6, :])
        nc.sync.dma_start(out=rxt_sb[:, 6:8, :], in_=rxt[:, 6:8, :])
        nc.sync.dma_start(out=wv_sb[:], in_=wv)
        nc.sync.dma_start(out=rwv_sb[:], in_=rwv)
        nc.sync.dma_start(out=wq_sb[:, 1], in_=wq[:, 1])
        nc.sync.dma_start(out=wk_sb[:, 1], in_=wk[:, 1])
        nc.sync.dma_start(out=rwq_sb[:, 1], in_=rwq[:, 1])
        nc.sync.dma_start(out=rwk_sb[:, 1], in_=rwk[:, 1])
        nc.sync.dma_start(out=masks_sb[:], in_=masks)
        nc.sync.dma_start(out=oneh_sb[:], in_=oneh)
        bo_b = bass.AP(tensor=bo.tensor, offset=bo.offset, ap=[[0, 128], [1, D]])
        nc.sync.dma_start(out=bo_sb[:], in_=bo_b)
        # ones column of v4 via memset (strided view)
        nc.gpsimd.memset(
            v4[:].rearrange("p t (h c) -> p t h c", c=65)[:, :, :, 64:65], 1.0
        )

        y_pool = ctx.enter_context(tc.tile_pool(name="y", bufs=6))
        rc_pool = ctx.enter_context(tc.tile_pool(name="rc", bufs=4))
        an_pool = ctx.enter_context(tc.tile_pool(name="an", bufs=4))
        pt_pool = ctx.enter_context(tc.tile_pool(name="pt", bufs=3))

        # ---- P1 pair 0: Q/K fp8 DoubleRow 3-term (x8.W8 + rx8.W8 + x8.rW8)
        # with 8 live psum accumulators, then V (same 3-term scheme)
        qk_terms = (
            (xt_sb, wq_sb, wk_sb),
            (rxt_sb, wq_sb, wk_sb),
            (xt_sb, rwq_sb, rwk_sb),
        )
        with ExitStack() as scope1:
            ps1 = scope1.enter_context(tc.tile_pool(name="ps1", bufs=2, space="PSUM"))
            psqk = [
                ps1.tile([128, 512], F32, tag=f"qk{i}", name=f"qk{i}", bufs=1)
                for i in range(8)
            ]
            for term, (xsrc, wq_t, wk_t) in enumerate(qk_terms):
                for cp in range(4):
                    for i, w_sb in ((0, wq_t), (4, wk_t)):
                        for nb in range(QB):
                            nc.tensor.matmul(
                                psqk[i + nb][:],
                                w_sb[:, 0, 2 * cp : 2 * cp + 2, :],
                                xsrc[:, 2 * cp : 2 * cp + 2, bass.ts(nb, 512)],
                                start=(term == 0 and cp == 0),
                                stop=(term == 2 and cp == 3),
                                perf_mode=DR,
                            )
            for i, dst in ((0, qst[0]), (4, kst[0])):
                for nb in range(QB):
                    nc.vector.tensor_copy(dst[:, bass.ts(nb, 512)], psqk[i + nb][:])
            nc.sync.dma_start(out=qtb[0][:], in_=qst[0][64:128, :])
            nc.sync.dma_start(out=ktb[0][:], in_=kst[0][64:128, :])
            nc.sync.dma_start(out=wo2_sb[:], in_=wo2)
            # V for all 4 heads (st-outer, 3-term fp8 ct accumulation)
            for st in range(KT):
                ps = ps1.tile([128, 256], F32, tag=f"qk{st % 8}", name="psv", bufs=1)
                for term, (xsrc, _, _) in enumerate(qk_terms):
                    wv_t = rwv_sb if term == 2 else wv_sb
                    for cp in range(4):
                        nc.tensor.matmul(
                            ps[:],
                            xsrc[:, 2 * cp : 2 * cp + 2, bass.ts(st, 128)],
                            wv_t[:, 2 * cp : 2 * cp + 2, :],
                            start=(term == 0 and cp == 0),
                            stop=(term == 2 and cp == 3),
                            perf_mode=DR,
                        )
                nc.scalar.activation(
                    v4[:, st, :].rearrange("p (h c) -> p h c", c=65)[:, :, 0:64],
                    ps[:].rearrange("p (h c) -> p h c", c=64),
                    mybir.ActivationFunctionType.Copy,
                    scale=1.0 / WS,
                )

        # ---- P2: attention, software-pipelined, with interleaved fillers
        ps2 = ctx.enter_context(tc.tile_pool(name="ps2", bufs=2, space="PSUM"))

        def attn_scores(p, j, t, qv):
            """scores^T both heads -> exp -> mask (diagonal). Returns pt2."""
            r = t - 4 * j
            c0 = 128 * r if r > 0 else 0
            pss = ps2.tile([128, 1024], F32, tag="pss", name="pss", bufs=3)
            for q in range(2):
                qt, kt = qv[q]
                nc.tensor.matmul(
                    pss[:, 512 * q + c0 : 512 * (q + 1)],
                    kt[:, bass.ts(t, 128)],
                    qt[:, 512 * j + c0 : 512 * (j + 1)],
                    start=True,
                    stop=True,
                )
            pt2 = pt_pool.tile([128, 1024], BF16, tag="pt2", name="pt2")
            if r < 0:
                nc.scalar.activation(pt2[:], pss[:], EXP, scale=SCALE)
            else:
                pv = pss[:].rearrange("p (h c) -> p h c", c=512)[:, :, c0:512]
                ov = pt2[:].rearrange("p (h c) -> p h c", c=512)[:, :, c0:512]
                nc.scalar.activation(ov, pv, EXP, scale=SCALE)
                mv = pt2[:].rearrange("p (h c) -> p h c", c=512)[:, :, c0 : c0 + 128]
                nc.vector.tensor_mul(mv, mv, masks_sb[:])
            return pt2

        def attn_pv(p, j, t, pt2, psa):
            """PV in [q, hd] orientation: lhsT = P^T tile (stationary 128 q),
            rhs = V (+ones col) moving 65. psa[q] = [128 q, 4 qt x 65]."""
            r = t - 4 * j
            for q in range(2):
                h = 2 * p + q
                vsl = v4[:, t, bass.ds(65 * h, 65)]
                for qt in range(max(r, 0), 4):
                    # start only on the bank's very first matmul: start=True
                    # clears the has_written bits of the WHOLE 2KB zero
                    # region, so a second start would wipe sibling qt
                    # regions' accumulate state
                    nc.tensor.matmul(
                        psa[q][:, 65 * qt : 65 * qt + 65],
                        pt2[:, 512 * q + 128 * qt : 512 * q + 128 * qt + 128],
                        vsl,
                        start=(t == 0 and qt == 0),
                        stop=(t == 4 * j + qt),
                    )

        def norm_transpose(p, j, psa, shift_by_copy=False, keepers=False):
            if DEBUG_DUMP and p == 0 and j == 0:
                dbg2_sb = a_pool.tile([128, 260], F32, tag="dbg2sb")
                nc.vector.tensor_copy(dbg2_sb[:], psa[0][:])
                nc.sync.dma_start(out=dbg2, in_=dbg2_sb[:])
            """Reciprocal of the softmax denominators (col 64 of each 65-block
            of psa), per-partition-scalar normalize psum->sbuf bf16, PE
            transpose back to [hd, q], land both halves in attnT2b.
            shift_by_copy: emit the odd-m shifted duplicate as a second
            psum->sbuf copy instead of a SBUF DMA (used in the tail where the
            DMA's ~1.6us latency would sit on the critical path)."""
            rcps, ans, psts = [], [], []
            if keepers:
                # immediate ramp-keepers cover the recip latency window
                dmy = ps2.tile([128, 512], F32, tag="pss", name="dmy", bufs=3)
                for _ in range(4):
                    nc.tensor.matmul(
                        dmy[0:4, 0:128], oneh_sb[:, 0:4], oneh_sb[:],
                        start=True, stop=True,
                    )
            for q in range(2):
                rcp = rc_pool.tile([128, 4], F32, tag="rc", name="rc")
                dn = psa[q][:].rearrange("p (qt c) -> p qt c", c=65)[:, :, 64:65]
                with nc.allow_low_precision(reason="softmax denom reciprocal"):
                    nc.vector.reciprocal(rcp[:], dn)
                rcps.append(rcp)
            for q in range(2):
                an = an_pool.tile([128, 256], BF16, tag="an", name="an")
                for qt in range(4):
                    nc.vector.tensor_scalar_mul(
                        an[:, 64 * qt : 64 * qt + 64],
                        psa[q][:, 65 * qt : 65 * qt + 64],
                        rcps[q][:, qt : qt + 1],
                    )
                ans.append(an)
                if keepers and q == 0:
                    # gated keepers: ready right as the q0 muls complete,
                    # keeping PE warm until the transposes are unblocked
                    for _ in range(6):
                        nc.tensor.matmul(
                            dmy[0:4, 0:128], an[:, 0:4], ident[:],
                            start=True, stop=True,
                        )
            for q in range(2):
                rows = 128 if shift_by_copy else 64
                pst = ps2.tile([rows, 512], BF16, tag=f"psa{q}", name=f"pst{q}", bufs=1)
                for qt in range(4):
                    nc.tensor.transpose(
                        pst[0:64, 128 * qt : 128 * qt + 128],
                        ans[q][:, 64 * qt : 64 * qt + 64],
                        ident[:],
                    )
                    if shift_by_copy:
                        # shifted duplicate straight from a second transpose
                        # (within-tile shift; odd cols 127/255/... land as 0,
                        # never read by the proj's even-m access pattern)
                        nc.tensor.transpose(
                            pst[64:128, 128 * qt : 128 * qt + 128],
                            ans[q][:, 64 * qt : 64 * qt + 64],
                            ident_s[:],
                        )
                psts.append(pst)
            for q in range(2):
                h = 2 * p + q
                if shift_by_copy:
                    nc.vector.tensor_copy(
                        attnT2b[h][:, bass.ts(j, 512)], psts[q][:]
                    )
                else:
                    nc.vector.tensor_copy(
                        attnT2b[h][0:64, bass.ts(j, 512)], psts[q][:]
                    )
                    nc.sync.dma_start(
                        out=attnT2b[h][64:128, 512 * j : 512 * j + 511],
                        in_=attnT2b[h][0:64, 512 * j + 1 : 512 * (j + 1)],
                    )

        def attn_j(p, j, qv, slot0=(), slot1=(), slot2=(), pre=None):
            """Skewed pipeline: S0 S1 [pre] [slot0] P0 S2 [slot1] P1 S3 ... PT.
            Each slot must stay under ~2us of PE work (the Act runway from
            the two queued exps) or Act starves behind the in-order PE queue.
            Returns a closure finishing this j (normalize+transpose), which
            the caller passes as `pre` to the NEXT attn_j so the next block's
            first scores are not queued behind it."""
            T = 4 * j + 4
            pts = [attn_scores(p, j, 0, qv)]
            if T > 1:
                pts.append(attn_scores(p, j, 1, qv))
            if pre is not None:
                pre()
            # psa allocated AFTER pre(): the previous j's pst tiles share the
            # psa tags, so the per-tag rotation must see psa(j-1), pst(j-1),
            # psa(j) in that order to stay acyclic
            psa = [
                ps2.tile([128, 260], F32, tag=f"psa{q}", name=f"psa{q}", bufs=1)
                for q in range(2)
            ]
            for f in slot0:
                f()
            for t in range(T):
                if t + 2 < T:
                    pts.append(attn_scores(p, j, t + 2, qv))
                if t == 1:
                    for f in slot1:
                        f()
                if t == 5:
                    for f in slot2:
                        f()
                attn_pv(p, j, t, pts[t], psa)

            def finish():
                norm_transpose(p, j, psa)

            return finish, psa

        def qk1_half(half, nb):
            """Pair-1 Q (half=0) or K (half=1) fp8 3-term projection for one
            512-col s-block (rides pss tag; ~1.3us of PE)."""
            w_sb, rw_sb, dst, dstb = (
                (wq_sb, rwq_sb, qst[1], qtb[1]),
                (wk_sb, rwk_sb, kst[1], ktb[1]),
            )[half]
            pssqk = ps2.tile([128, 1024], F32, tag="pss", name="pssqk", bufs=3)
            for term, (xsrc, wt) in enumerate(
                ((xt_sb, w_sb), (rxt_sb, w_sb), (xt_sb, rw_sb))
            ):
                for cp in range(4):
                    nc.tensor.matmul(
                        pssqk[:, 0:512],
                        wt[:, 1, 2 * cp : 2 * cp + 2, :],
                        xsrc[:, 2 * cp : 2 * cp + 2, bass.ts(nb, 512)],
                        start=(term == 0 and cp == 0),
                        stop=(term == 2 and cp == 3),
                        perf_mode=DR,
                    )
            nc.vector.tensor_copy(dst[:, bass.ts(nb, 512)], pssqk[:, 0:512])
            nc.sync.dma_start(
                out=dstb[:, bass.ts(nb, 512)], in_=dst[64:128, bass.ts(nb, 512)]
            )

        def proj_qq(h, qq0, tail=False):
            """Project head h for quarter pair (qq0, qq0+1); one psum tile
            holds both quarters so the pss-tag rotation stalls half as often.
            In the tail, ys adds and y DMA issues are spread across engines
            so the final store ladder does not serialize on one queue."""
            a2 = attnT2b[h][:].rearrange("p (r s) -> p s r", s=16)
            psy = ps2.tile([128, 1024], F32, tag="pss", name="psy", bufs=3)
            for k in range(2):
                qq = qq0 + k
                if tail and k == 1:
                    # pre-add bo into psum so the psum->sbuf move can be an
                    # Act-engine Copy (Act is idle in the tail; DVE is not)
                    nc.tensor.matmul(
                        psy[:, bass.ts(k, 256)],
                        ones_col[:],
                        bo_sb[0:1, bass.ts(qq, 256)],
                        start=True,
                        stop=False,
                    )
                for mp in range(8):
                    nc.tensor.matmul(
                        psy[:, bass.ts(k, 256)],
                        a2[:, 2 * mp, :],
                        wo2_sb[:, mp, qq, :],
                        start=(mp == 0) and not (tail and k == 1),
                        stop=(mp == 7),
                    )
            for k in range(2):
                qq = qq0 + k
                ys = y_pool.tile([128, 256], F32, tag="ys", name="ys")
                if tail and k == 1:
                    nc.scalar.activation(
                        ys[:], psy[:, bass.ts(k, 256)],
                        mybir.ActivationFunctionType.Copy,
                    )
                else:
                    nc.vector.tensor_add(
                        ys[:], psy[:, bass.ts(k, 256)], bo_sb[:, bass.ts(qq, 256)]
                    )
                nc.sync.dma_start(out=y[bass.ts(h, 128), bass.ts(qq, 256)], in_=ys[:])

        qv0 = [(qst[0][0:64, :], kst[0][0:64, :]), (qtb[0][:], ktb[0][:])]
        qv1 = [(qst[1][0:64, :], kst[1][0:64, :]), (qtb[1][:], ktb[1][:])]

        def Q1(nb):
            return lambda: qk1_half(0, nb)

        def K1(nb):
            return lambda: qk1_half(1, nb)

        def P(h, qq):
            return lambda: proj_qq(h, qq)

        # qk1 half-chunks (~1.3us each) are deadline-scheduled: attn1-j needs
        # qst[1]/kst[1] block nb=j only. Pair-0 projections fill pair-1's
        # attention (attnT2b[0..1] complete after finish(0,3) = pre of (1,0)).
        # Each j's finish (normalize+transpose) is deferred into the next j,
        # emitted behind its first two scores so the Act feed isn't delayed.
        fin, _ = attn_j(0, 0, qv0, [Q1(0)])
        fin, _ = attn_j(0, 1, qv0, [K1(0)], pre=fin)
        fin, _ = attn_j(0, 2, qv0, [Q1(1)], [K1(1)], pre=fin)
        fin, _ = attn_j(0, 3, qv0, [Q1(2)], [K1(2)], pre=fin)

        fin, _ = attn_j(1, 0, qv1, [Q1(3)], [P(0, 0)], pre=fin)
        fin, _ = attn_j(1, 1, qv1, [K1(3)], [P(0, 2)], pre=fin)
        fin, _ = attn_j(1, 2, qv1, [P(1, 0)], [P(1, 2)], pre=fin)
        fin, psa13 = attn_j(1, 3, qv1, pre=fin)

        # tail: finish pair-1 last j inline (shifted dup via psum copies so no
        # DMA latency gates the projections), then heads 2,3 projections
        norm_transpose(1, 3, psa13, shift_by_copy=True, keepers=True)
        proj_qq(2, 0, tail=True)
        proj_qq(2, 2, tail=True)
        proj_qq(3, 0, tail=True)
        for qq in (2, 3):
            a2 = attnT2b[3][:].rearrange("p (r s) -> p s r", s=16)
            psy = ps2.tile([128, 1024], F32, tag="pss", name="psy1", bufs=3)
            for mp in range(8):
                nc.tensor.matmul(
                    psy[:, 0:256],
                    a2[:, 2 * mp, :],
                    wo2_sb[:, mp, qq, :],
                    start=(mp == 0),
                    stop=(mp == 7),
                )
            ys = y_pool.tile([128, 256], F32, tag="ys", name="ys")
            eng_d = (nc.scalar, nc.sync)[qq - 2]
            nc.vector.tensor_add(ys[:], psy[:, 0:256], bo_sb[:, bass.ts(qq, 256)])
            eng_d.dma_start(out=y[bass.ts(3, 128), bass.ts(qq, 256)], in_=ys[:])
        if DEBUG_DUMP:
            nc.sync.dma_start(out=dbg, in_=attnT2b[0][:])

    nc.compile()
    return nc


def make_masks():
    kl = np.arange(128)[:, None]
    cl = np.arange(128)[None, :]
    tri = (kl <= cl).astype(NPBF16)  # [128 k, 128 c]
    return np.ascontiguousarray(np.stack([tri, tri], 1))  # [128, 2, 128]


def fp8_split(a):
    """a (f32) -> (fp8(a), fp8(a - fp8(a))) both contiguous."""
    a8 = a.astype(NPFP8)
    r8 = (a - a8.astype(np.float32)).astype(NPFP8)
    return np.ascontiguousarray(a8), np.ascontiguousarray(r8)


def prep_core_inputs(c, x, Wq, Wk, Wv, Wo, bo):
    b, g = c // 4, c % 4
    heads = [4 * g + i for i in range(HPC)]
    xt, rxt = fp8_split(x[b].T.reshape(CT, 128, S).transpose(1, 0, 2))

    def pack_pair(W, p):
        h0, h1 = heads[2 * p], heads[2 * p + 1]
        cols = np.concatenate(
            [W[:, 64 * h0 : 64 * h0 + 64], W[:, 64 * h1 : 64 * h1 + 64]], 1
        )
        return cols.reshape(CT, 128, 128)

    wq, rwq = fp8_split(
        np.stack([pack_pair(Wq, p) for p in range(2)]).transpose(2, 0, 1, 3) * WS
    )  # [128, 2, CT, 128]
    wk, rwk = fp8_split(
        np.stack([pack_pair(Wk, p) for p in range(2)]).transpose(2, 0, 1, 3) * WS
    )
    wv, rwv = fp8_split(
        np.concatenate([Wv[:, 64 * h : 64 * h + 64] for h in heads], 1)
        .reshape(CT, 128, 256)
        .transpose(1, 0, 2)
        * WS
    )  # [128, CT, 256]
    # wo2[d, mp, qq, :] = Wo[128*mp + d, 256*qq : 256*(qq+1)]
    wo2 = np.ascontiguousarray(
        Wo.reshape(8, 128, 4, 256).transpose(1, 0, 2, 3)
    ).astype(NPBF16)  # [128, 8, 4, 256]
    oneh = np.kron(np.eye(2, dtype=np.float32), np.ones((1, 64), np.float32)).astype(
        NPBF16
    )  # [2, 128]
    return {
        "xt": xt,
        "rxt": rxt,
        "wq": wq,
        "rwq": rwq,
        "wk": wk,
        "rwk": rwk,
        "wv": wv,
        "rwv": rwv,
        "wo2": wo2,
        "bo": bo.astype(NPBF16),
        "masks": make_masks(),
        "oneh": oneh,
    }


_NC_CACHE = []


def kernel(x, Wq, Wk, Wv, Wo, bo):
    from concourse import bass_utils

    x, Wq, Wk, Wv, Wo, bo = (
        np.asarray(x, np.float32),
        np.asarray(Wq, np.float32),
        np.asarray(Wk, np.float32),
        np.asarray(Wv, np.float32),
        np.asarray(Wo, np.float32),
        np.asarray(bo, np.float32),
    )
    if not _NC_CACHE:
        _NC_CACHE.append(build_nc())
    nc = _NC_CACHE[0]
    in_maps = [prep_core_inputs(c, x, Wq, Wk, Wv, Wo, bo) for c in range(NC)]
    res = bass_utils.run_bass_kernel_spmd(nc, in_maps, core_ids=list(range(NC)))
    out = np.empty((B, S, D), np.float32)
    for c in range(NC):
        b, g = c // 4, c % 4
        out[b, 512 * g : 512 * (g + 1), :] = res.results[c]["y"]
    return out

